# revision 15
# baseline (speedup 1.0000x reference)
"""GAT-style dense-mask attention (gnn_message_passing) on 8 trn2 cores.

Sharding (hpc=2): core c owns query rows [r0, r0+1024), r0=(c//2)*1024, and
head pair hp=c%2 (heads 2hp, 2hp+1). The adjacency slice [4096, 1024] is
shared by both heads, so adj DMA is 4.2 MB/core (vs 16.8 at one head/core).
Inputs are pre-sliced/rolled on host (own rows first); all 8 cores run one
identical SPMD program; outputs are [1024, 256] blocks reassembled on host.

Math per core (j = source node on partitions, i = own query row on free dim):
  s[j,i] = f1[j] + f2[i],  p = adj * exp(prelu_0.2(s))
  out[i,:] = (p.T @ [supp|1])[:, :128] / (...)[:, 128] + res
Two ways to produce p, statically assigned per (head, j-chunk) to balance
engines:
  P_A (ACT): m = Prelu(F2b + f1col); p = Exp(m); p *= adj   (mask on DVE/GPS)
  P_D (DVE): exp(prelu(s)) = max(e1[j]E1[i], e2[j]E2[i]) with e1=exp(f1),
     e2=exp(0.2 f1), E1=exp(f2), E2=exp(0.2 f2) (exact: exp is monotone, so
     max(e^s, e^{0.2s}) = e^{max(s,0.2s)}). Three DVE ops:
       V = adj*E2b; q1 = stt(adj, e1col, mult, E1b, mult);
       p = stt(V, e2col, mult, q1, max)
  P_M: like P_D but V computed on gpsimd.
The aggregation accumulates all 32 j-chunks of each output tile in a
persistent PSUM bank (start/stop flags span the whole stage 2), so there is
no SBUF accumulation pass at all; the epilogue reads PSUM directly.
"""

import os

import ml_dtypes
import numpy as np

N = 4096
IN = 512
D = 128
H = 4
NCORES = 8
RPC = 1024            # query rows per core
HPC = 2               # heads per core
JCH = N // 128        # 32 source-node chunks
ICH = RPC // 128      # 8 query-row chunks

_cache = {}


def _path_map():
    """Static per-(head-local, j-chunk) path assignment.
    Pattern per 8 chunks: 5x P_A (masks alternating DVE/GPS), 2x P_D, 1x P_M.
    Overridable via KERNEL_PATTERN (string of A/D/M of length 8)."""
    pat = os.environ.get("KERNEL_PATTERN", "AADAAMDA")
    assert len(pat) == 8 and set(pat) <= set("ADM")
    paths = {}
    for h in range(HPC):
        for jc in range(JCH):
            paths[(h, jc)] = pat[jc % 8]
    return paths


def _build_program():
    import concourse.bacc as bacc
    import concourse.mybir as mybir
    import concourse.tile as tile

    f32 = mybir.dt.float32
    bf16 = mybir.dt.bfloat16
    Prelu = mybir.ActivationFunctionType.Prelu
    Exp = mybir.ActivationFunctionType.Exp
    Copy = mybir.ActivationFunctionType.Copy
    add = mybir.AluOpType.add
    mult = mybir.AluOpType.mult
    mx = mybir.AluOpType.max

    paths = _path_map()
    gps_mask = os.environ.get("KERNEL_GPS_MASK", "1") == "1"
    supp_cp_gps = os.environ.get("KERNEL_SUPP_GPS", "1") == "1"

    nc = bacc.Bacc(
        "TRN2",
        target_bir_lowering=False,
        debug=False,
        enable_asserts=False,
        num_devices=NCORES,
    )

    debug = os.environ.get("KERNEL_DEBUG", "0") == "1"
    adjT = nc.dram_tensor("adjT", [N, RPC], bf16, kind="ExternalInput").ap()
    inpT = nc.dram_tensor("inpT", [IN, N], bf16, kind="ExternalInput").ap()
    # rhs columns for the fused stage-1 pass: [w1h0 w2h0 w1h1 w2h1 | Wh0 | Wh1]
    rhsW = nc.dram_tensor("rhsW", [IN, 260], bf16, kind="ExternalInput").ap()
    pwh = nc.dram_tensor("pwh", [IN, HPC * D], bf16, kind="ExternalInput").ap()
    br = nc.dram_tensor("br", [1, HPC * D], f32, kind="ExternalInput").ap()
    outb = nc.dram_tensor("outb", [RPC, HPC * D], f32, kind="ExternalOutput").ap()
    if debug:
        dbg = {
            "supp": nc.dram_tensor(
                "dbg_supp", [128, JCH * 258], bf16,
                kind="ExternalOutput").ap(),
            "f12s": nc.dram_tensor(
                "dbg_f12s", [128, JCH * 4], f32, kind="ExternalOutput").ap(),
            "E1b0": nc.dram_tensor(
                "dbg_E1b0", [128, RPC], bf16, kind="ExternalOutput").ap(),
            "E2b0": nc.dram_tensor(
                "dbg_E2b0", [128, RPC], bf16, kind="ExternalOutput").ap(),
            "F2b0": nc.dram_tensor(
                "dbg_F2b0", [128, RPC], bf16, kind="ExternalOutput").ap(),
            "e1c0": nc.dram_tensor(
                "dbg_e1c0", [128, JCH], f32, kind="ExternalOutput").ap(),
            "pA": nc.dram_tensor(
                "dbg_pA", [128, RPC], bf16, kind="ExternalOutput").ap(),
            "pD": nc.dram_tensor(
                "dbg_pD", [128, RPC], bf16, kind="ExternalOutput").ap(),
            "acc00": nc.dram_tensor(
                "dbg_acc00", [128, 3 * 129], f32, kind="ExternalOutput").ap(),
        }

    with tile.TileContext(nc) as tc:
        with tc.tile_pool(name="persist", bufs=1) as persist:
            # supp_all: per j-chunk [h0(128)|1|h1(128)|1]; ones col via memset
            supp_all = persist.tile([128, JCH * 258], bf16)
            f12s = persist.tile([128, JCH * 4], f32)     # w12 cols per chunk
            e1c = [persist.tile([128, JCH], f32, name=f"e1c{h}")
                   for h in range(HPC)]
            e2c = [persist.tile([128, JCH], f32, name=f"e2c{h}")
                   for h in range(HPC)]
            F2b = [persist.tile([128, RPC], bf16, name=f"F2b{h}")
                   for h in range(HPC)]
            E1b = [persist.tile([128, RPC], bf16, name=f"E1b{h}")
                   for h in range(HPC)]
            E2b = [persist.tile([128, RPC], bf16, name=f"E2b{h}")
                   for h in range(HPC)]
            res = [persist.tile([128, HPC * D], f32, name=f"res{ic}")
                   for ic in range(ICH)]
            alpha_col = persist.tile([128, 1], f32)
            bias_bc = persist.tile([128, HPC * D], f32)
            nc.vector.memset(supp_all, 1.0)
            nc.vector.memset(alpha_col, 0.2)

            # ---- stage 1 ----
            with tc.tile_pool(name="s1c", bufs=1) as s1c, \
                 tc.tile_pool(name="s1p", bufs=2, space="PSUM") as s1p, \
                 tc.tile_pool(name="s1pp", bufs=2, space="PSUM") as s1pp, \
                 tc.tile_pool(name="s1f2", bufs=2, space="PSUM") as s1f2, \
                 tc.tile_pool(name="s1in", bufs=2) as s1in:
                rhs_sb = []
                for kc in range(4):
                    t = s1c.tile([128, 260], bf16, tag=f"rhs{kc}")
                    nc.scalar.dma_start(
                        out=t, in_=rhsW[kc * 128:(kc + 1) * 128, :])
                    rhs_sb.append(t)
                pwh_sb = []
                for kc in range(4):
                    t = s1c.tile([128, HPC * D], bf16, tag=f"pwh{kc}")
                    nc.scalar.dma_start(
                        out=t, in_=pwh[kc * 128:(kc + 1) * 128, :])
                    pwh_sb.append(t)
                br_sb = s1c.tile([1, HPC * D], f32)
                nc.scalar.dma_start(out=br_sb, in_=br)
                nc.gpsimd.partition_broadcast(bias_bc, br_sb)

                # own-block inputs first (feed early f2 + first m-chunks)
                it_blks = {}
                for blk in range(4):
                    it_blks[blk] = []
                    for kc in range(4):
                        t = s1in.tile([128, 1024], bf16, tag=f"it{kc}")
                        nc.sync.dma_start(
                            out=t,
                            in_=inpT[kc * 128:(kc + 1) * 128,
                                     blk * 1024:(blk + 1) * 1024])
                        it_blks[blk].append(t)
                    if blk == 0:
                        # early f2 per head: [1, 1024] = w2_h.T @ X_own
                        # (single-row matmuls so every row sits at partition 0
                        #  — engines cannot read APs at a partition offset)
                        for h in range(HPC):
                            f2r = s1c.tile([1, RPC], f32, tag=f"f2r{h}",
                                           name=f"f2r{h}")
                            for half in range(2):
                                fp = s1f2.tile([1, 512], f32, tag="f2ps")
                                for kc in range(4):
                                    nc.tensor.matmul(
                                        fp,
                                        rhs_sb[kc][:, 1 + 2 * h:2 + 2 * h],
                                        it_blks[0][kc][:, half * 512:
                                                       (half + 1) * 512],
                                        start=(kc == 0), stop=(kc == 3),
                                    )
                                nc.scalar.copy(
                                    out=f2r[:, half * 512:(half + 1) * 512],
                                    in_=fp)
                            f2rb = s1c.tile([1, RPC], bf16, tag=f"f2rb{h}",
                                            name=f"f2rb{h}")
                            e1r = s1c.tile([1, RPC], bf16, tag=f"e1r{h}",
                                           name=f"e1r{h}")
                            e2r = s1c.tile([1, RPC], bf16, tag=f"e2r{h}",
                                           name=f"e2r{h}")
                            nc.scalar.copy(out=f2rb, in_=f2r)
                            nc.scalar.activation(e1r, f2r, Exp, scale=1.0)
                            nc.scalar.activation(e2r, f2r, Exp, scale=0.2)
                            nc.gpsimd.partition_broadcast(F2b[h], f2rb)
                            nc.gpsimd.partition_broadcast(E1b[h], e1r)
                            nc.gpsimd.partition_broadcast(E2b[h], e2r)

                    # main pass for this block's 8 m-chunks
                    for mj in range(8):
                        jc = blk * 8 + mj
                        ps = s1p.tile([128, 260], f32, tag="ps")
                        for kc in range(4):
                            lhsT = it_blks[blk][kc][:, mj * 128:(mj + 1) * 128]
                            nc.tensor.matmul(
                                ps, lhsT, rhs_sb[kc],
                                start=(kc == 0), stop=(kc == 3),
                            )
                        # supp copy: [w12 skipped] cols 4..260 -> two 128-wide
                        # blocks at jc*258 (+0 and +129), ones cols preserved
                        so = supp_all[:, jc * 258:(jc + 1) * 258].rearrange(
                            "p (c w) -> p c w", c=2)[:, :, 0:128]
                        psv = ps[:, 4:260].rearrange("p (c w) -> p c w", c=2)
                        # gpsimd cannot read PSUM; alternate ACT/DVE drains
                        if supp_cp_gps and jc % 2 == 0:
                            nc.scalar.copy(out=so, in_=psv)
                            nc.vector.tensor_copy(
                                out=f12s[:, jc * 4:(jc + 1) * 4],
                                in_=ps[:, 0:4])
                        else:
                            nc.vector.tensor_copy(out=so, in_=psv)
                            nc.scalar.copy(
                                out=f12s[:, jc * 4:(jc + 1) * 4],
                                in_=ps[:, 0:4])
                        if jc < ICH:
                            # residual projection for own rows
                            ps2 = s1pp.tile([128, HPC * D], f32, tag="ps2")
                            for kc in range(4):
                                lhsT = it_blks[blk][kc][:, mj * 128:
                                                        (mj + 1) * 128]
                                nc.tensor.matmul(
                                    ps2, lhsT, pwh_sb[kc],
                                    start=(kc == 0), stop=(kc == 3),
                                )
                            nc.vector.tensor_add(res[jc], ps2, bias_bc)
                    # e-cols for this block's chunks (gates stage-2 P_D)
                    f12v = f12s.rearrange("p (c k) -> p c k", k=4)
                    sl = slice(blk * 8, (blk + 1) * 8)
                    for h in range(HPC):
                        nc.scalar.activation(
                            e1c[h][:, sl], f12v[:, sl, 2 * h], Exp, scale=1.0)
                        nc.scalar.activation(
                            e2c[h][:, sl], f12v[:, sl, 2 * h], Exp, scale=0.2)

            if debug:
                nc.sync.dma_start(out=dbg["supp"], in_=supp_all)
                nc.sync.dma_start(out=dbg["f12s"], in_=f12s)
                nc.sync.dma_start(out=dbg["E1b0"], in_=E1b[0])
                nc.sync.dma_start(out=dbg["E2b0"], in_=E2b[0])
                nc.sync.dma_start(out=dbg["F2b0"], in_=F2b[0])
                nc.sync.dma_start(out=dbg["e1c0"], in_=e1c[0])

            # ---- stage 2 ----
            with tc.tile_pool(name="adjp", bufs=6) as adjp, \
                 tc.tile_pool(name="mtp", bufs=3) as mtp, \
                 tc.tile_pool(name="pbufp", bufs=10) as pbufp, \
                 tc.tile_pool(name="tmpp", bufs=4) as tmpp, \
                 tc.tile_pool(name="epp", bufs=4) as epp, \
                 tc.tile_pool(name="accp", bufs=1, space="PSUM") as accp:
                # persistent psum accumulators: per head, blocks of ic-chunks
                ic_blocks = [[0, 1, 2], [3, 4, 5], [6, 7]]
                acc = {}
                for h in range(HPC):
                    for b, icb in enumerate(ic_blocks):
                        acc[(h, b)] = accp.tile(
                            [128, len(icb) * 129], f32, tag=f"acc{h}_{b}",
                            name=f"acc{h}_{b}")

                gps_mask_ctr = [0]

                def make_p(h, jc, adj_t, p_t, p_off):
                    """Produce p for (h, jc) into p_t[:, p_off:p_off+RPC]."""
                    kind = paths[(h, jc)]
                    psl = p_t[:, p_off:p_off + RPC]
                    f1col = f12s[:, jc * 4 + 2 * h:jc * 4 + 2 * h + 1]
                    if kind == "A":
                        m_t = mtp.tile([128, RPC], f32, tag="m")
                        nc.scalar.activation(
                            m_t, F2b[h], Prelu, bias=f1col, scale=1.0,
                            alpha=alpha_col[:, 0:1])
                        nc.scalar.activation(psl, m_t, Exp)
                        gps_mask_ctr[0] += 1
                        eng = nc.gpsimd if (gps_mask and
                                            gps_mask_ctr[0] % 2 == 0) \
                            else nc.vector
                        eng.tensor_mul(psl, adj_t, psl)
                    else:
                        v_t = tmpp.tile([128, RPC], bf16, tag="v")
                        eng = nc.gpsimd if kind == "M" else nc.vector
                        eng.tensor_mul(v_t, adj_t, E2b[h])
                        q_t = tmpp.tile([128, RPC], bf16, tag="q")
                        nc.vector.scalar_tensor_tensor(
                            q_t, in0=adj_t, scalar=e1c[h][:, jc:jc + 1],
                            in1=E1b[h], op0=mult, op1=mult)
                        nc.vector.scalar_tensor_tensor(
                            psl, in0=v_t, scalar=e2c[h][:, jc:jc + 1],
                            in1=q_t, op0=mult, op1=mx)

                NG = 8            # j-chunk groups of 4
                for g in range(NG):
                    jcs = list(range(g * 4, (g + 1) * 4))
                    adj_ts = {}
                    for jc in jcs:
                        a_t = adjp.tile([128, RPC], bf16, tag="adj")
                        nc.sync.dma_start(
                            out=a_t, in_=adjT[jc * 128:(jc + 1) * 128, :])
                        adj_ts[jc] = a_t
                    p_ts = {}
                    for h in range(HPC):
                        for jc in jcs:
                            p_t = pbufp.tile([128, RPC], bf16, tag="pbuf")
                            make_p(h, jc, adj_ts[jc], p_t, 0)
                            p_ts[(h, jc)] = p_t
                            if debug and h == 0 and jc == 0:
                                nc.sync.dma_start(out=dbg["pA"], in_=p_t)
                            if debug and h == 0 and jc == 2:
                                nc.sync.dma_start(out=dbg["pD"], in_=p_t)
                    for h in range(HPC):
                        for b, icb in enumerate(ic_blocks):
                            a_ps = acc[(h, b)]
                            for i3, ic in enumerate(icb):
                                for jj, jc in enumerate(jcs):
                                    # start/stop are PSUM-BANK granular: only
                                    # the very first matmul into this bank may
                                    # set start (it zeroes the whole bank),
                                    # and only the very last sets stop.
                                    nc.tensor.matmul(
                                        a_ps[:, i3 * 129:(i3 + 1) * 129],
                                        p_ts[(h, jc)][:, ic * 128:
                                                      (ic + 1) * 128],
                                        supp_all[:, jc * 258 + h * 129:
                                                 jc * 258 + h * 129 + 129],
                                        start=(g == 0 and i3 == 0
                                               and jj == 0),
                                        stop=(g == NG - 1
                                              and i3 == len(icb) - 1
                                              and jj == 3),
                                        skip_group_check=True,
                                    )
                    if g == NG - 1:
                        if debug:
                            acc_sb = epp.tile([128, 3 * 129], f32,
                                              tag="dbgacc", name="acc_sb")
                            nc.vector.tensor_copy(out=acc_sb, in_=acc[(0, 0)])
                            nc.sync.dma_start(out=dbg["acc00"], in_=acc_sb)
                        # epilogue straight from psum
                        for h in range(HPC):
                            for b, icb in enumerate(ic_blocks):
                                a_ps = acc[(h, b)]
                                nb = len(icb)
                                dn = epp.tile([128, nb], f32, tag="dn")
                                rc = epp.tile([128, nb], f32, tag="rc")
                                av = a_ps.rearrange(
                                    "p (c w) -> p c w", c=nb)
                                nc.vector.tensor_scalar_add(
                                    dn, av[:, :, 128], 1e-30)
                                nc.vector.reciprocal(rc, dn)
                                for i3, ic in enumerate(icb):
                                    of = epp.tile([128, D], f32, tag="of")
                                    nc.vector.scalar_tensor_tensor(
                                        of,
                                        in0=a_ps[:, i3 * 129:i3 * 129 + 128],
                                        scalar=rc[:, i3:i3 + 1],
                                        in1=res[ic][:, h * D:(h + 1) * D],
                                        op0=mult, op1=add)
                                    nc.sync.dma_start(
                                        out=outb[ic * 128:(ic + 1) * 128,
                                                 h * D:(h + 1) * D],
                                        in_=of)

    nc.compile()
    return nc


def _get_program():
    key = ("prog",
           os.environ.get("KERNEL_PATTERN", "AADAAMDA"),
           os.environ.get("KERNEL_GPS_MASK", "1"),
           os.environ.get("KERNEL_SUPP_GPS", "1"),
           os.environ.get("KERNEL_DEBUG", "0"))
    if key not in _cache:
        _cache[key] = _build_program()
    return _cache[key]


def kernel(inputs, adjacency, weight, weight_u, weight_v, bias, proj_w, proj_b):
    from concourse.bass_utils import run_bass_kernel_spmd

    inputs = np.asarray(inputs, np.float32)
    adjacency = np.asarray(adjacency, np.float32)
    weight = np.asarray(weight, np.float32)
    weight_u = np.asarray(weight_u, np.float32)
    weight_v = np.asarray(weight_v, np.float32)
    bias = np.asarray(bias, np.float32).reshape(1, H * D)
    proj_w = np.asarray(proj_w, np.float32)
    proj_b = np.asarray(proj_b, np.float32).reshape(H * D)

    nc = _get_program()

    bf = ml_dtypes.bfloat16
    # per-head w1 = Wh @ u, w2 = Wh @ v (parameter folding on host)
    w12 = np.empty((IN, 2 * H), np.float32)
    for h in range(H):
        wh = weight[:, h * D:(h + 1) * D]
        w12[:, 2 * h] = wh @ weight_u[h, :, 0]
        w12[:, 2 * h + 1] = wh @ weight_v[h, :, 0]

    in_maps = []
    for c in range(NCORES):
        r0 = (c // 2) * RPC
        hp = c % 2
        h0 = 2 * hp
        hs = slice(h0 * D, (h0 + 2) * D)
        rolled = np.roll(inputs, -r0, axis=0)
        inpT_c = np.ascontiguousarray(rolled.T).astype(bf)
        adjT_c = np.ascontiguousarray(
            np.roll(adjacency[r0:r0 + RPC, :], -r0, axis=1).T).astype(bf)
        rhsW_c = np.concatenate(
            [w12[:, 2 * h0:2 * h0 + 4], weight[:, hs]], axis=1).astype(bf)
        in_maps.append({
            "adjT": adjT_c,
            "inpT": inpT_c,
            "rhsW": np.ascontiguousarray(rhsW_c),
            "pwh": np.ascontiguousarray(proj_w[:, hs]).astype(bf),
            "br": np.ascontiguousarray(
                (bias[0, hs] + proj_b[hs]).reshape(1, 2 * D)),
        })

    trace = os.environ.get("KERNEL_TRACE", "0") == "1"
    results = run_bass_kernel_spmd(
        nc, in_maps, core_ids=list(range(NCORES)), trace=trace)
    _cache["last_results"] = results

    out = np.empty((N, H * D), np.float32)
    for c in range(NCORES):
        r0 = (c // 2) * RPC
        hp = c % 2
        h0 = 2 * hp
        out[r0:r0 + RPC, h0 * D:(h0 + 2) * D] = results.results[c]["outb"]
    return out
